# revision 23
# baseline (speedup 1.0000x reference)
"""Cross-attention (B=4, C=256, H=W=64) Trainium2 Bass kernel.

Math (per batch b), with t = target[b] : [C, N], r = reference[b], N = H*W:
    q = Wq t + bq ; k = Wk r + bk ; v = Wv r + bv
    attn = softmax(q^T k / sqrt(C), axis=j)
    out = v attn^T + t

Sharding: 8 cores = 4 batches x 2 query-halves. Each core handles its
query slice of t (NQ = 2048) and the full r of its batch.

Algebraic folds (all exact):
  * scores: q_i . k_j = t_i^T (Wq^T Wk) r_j + bq.(Wk r_j) + (Wq t_i).bk + bq.bk
    The last two terms are per-query constants -> cancel in softmax.
    So with M = Wq^T Wk and g = Wk^T bq:  s[i,j] ~ r_j . u_i  where
    u = M^T t + g.  M, g are precomputed on the host.
  * bv: softmax rows sum to 1, so v -> v + bv just adds bv to the output;
    the host adds it.
  * normalization: the device returns o[c,i] = sum_j v[c,j] exp(s_ij)
    and the bf16 exp-matrix E; the host divides by colsum(E) (the exact
    denominator the AV matmul consumed) and adds the residual.

Precision: scores run as a single fp8e4m3 DoubleRow matmul per key block
(256-wide contraction; score noise largely cancels through the softmax
ratio), everything downstream of exp stays bf16 with fp32 accumulation.

Device layouts (matmuls contract over the partition axis):
    u8       : [128, (bb, i)] fp8   scores rhs, pair dim = channel half
    r8_sb    : [128, (jb, c_hi, j)] fp8  scores stationary (DoubleRow)
    r_sb[cc][ch] : [128, 1024] bf16 x4   v-projection stationary operand
    v_sb     : [128, NJB*C] bf16  V^T per key block: [j in block, c]
    scores   : S^T[j_blk, (ic2, i)] in a [128, 1024] PSUM tile; one exp
               (ACT) per key block covering a PAIR of query chunks; the
               AV pass runs one key block behind so exp latency hides.
"""

import os
import sys

import numpy as np

try:
    import concourse.bass as _probe  # noqa: F401
except ImportError:
    for _p in ("/opt/trn_rl_repo", "/root/.axon_site/_ro/trn_rl_repo"):
        if os.path.isdir(_p) and _p not in sys.path:
            sys.path.insert(0, _p)

import ml_dtypes

import concourse.bacc as bacc
import concourse.mybir as mybir
import concourse.tile as tile
from concourse.bass_utils import run_bass_kernel_spmd

BF16 = mybir.dt.bfloat16
FP8 = mybir.dt.float8e4
F32 = mybir.dt.float32
NPBF16 = ml_dtypes.bfloat16
NPFP8 = ml_dtypes.float8_e4m3

B, C, H, W = 4, 256, 64, 64
N = H * W                 # 4096 key/value pixels per batch
NCORES = 8
NQ = (B * N) // NCORES    # 2048 query pixels per core
P = 128
CB = C // P               # 2 channel blocks
ICH = 512                 # query chunk (one PSUM bank of fp32)
NICH = NQ // ICH          # 4
NJB = N // P              # 32 key blocks
RCH = 1024                # r/t chunk width (per-chunk SBUF tiles)
SCALE = float(C) ** -0.5

# Set by test harness: trace=True to collect an NTFF profile.
TRACE = False
LAST_RESULTS = None


def _build():
    nc = bacc.Bacc("TRN2", target_bir_lowering=False, debug=False,
                   num_devices=NCORES)

    t = nc.dram_tensor("t", [C, NQ], BF16, kind="ExternalInput")
    r = nc.dram_tensor("r", [C, N], BF16, kind="ExternalInput")
    r8 = nc.dram_tensor("r8", [P, 2 * N], FP8, kind="ExternalInput")
    m = nc.dram_tensor("m", [C, C], BF16, kind="ExternalInput")
    wvT = nc.dram_tensor("wvT", [C, C], BF16, kind="ExternalInput")
    g = nc.dram_tensor("g", [C, 1], F32, kind="ExternalInput")
    o = nc.dram_tensor("o", [C, NQ], F32, kind="ExternalOutput")
    e_out = nc.dram_tensor("e_out", [N, NQ], BF16, kind="ExternalOutput")

    with tile.TileContext(nc) as tc:
        with (
            tc.tile_pool(name="persist", bufs=1) as persist,
            tc.tile_pool(name="epool", bufs=4) as epool,
            tc.tile_pool(name="outp", bufs=4) as outp,
            tc.tile_pool(name="ps_s", bufs=2, space="PSUM") as ps_s,
            tc.tile_pool(name="ps_av", bufs=4, space="PSUM") as ps_av,
        ):
            # ---- load inputs: constants first, then t (which gates the
            # u-projection and thereby everything), r8, then r chunks.
            t_sb, r_sb, m_sb, wv_sb, g_sb = [], [], [], [], []
            for cc in range(CB):
                cs = slice(cc * P, (cc + 1) * P)
                for lst, src, nm in ((m_sb, m, "m"), (wv_sb, wvT, "wv")):
                    wtile = persist.tile([P, C], BF16, tag=f"{nm}{cc}")
                    nc.sync.dma_start(out=wtile[:], in_=src[cs, :])
                    lst.append(wtile)
                gt = persist.tile([P, 1], F32, tag=f"g{cc}")
                nc.sync.dma_start(out=gt[:], in_=g[cs, :])
                g_sb.append(gt)
                r_sb.append([persist.tile([P, RCH], BF16, tag=f"r{cc}_{ch}",
                                          name=f"r{cc}_{ch}")
                             for ch in range(N // RCH)])
                t_sb.append([persist.tile([P, RCH], BF16, tag=f"t{cc}_{ch}",
                                          name=f"t{cc}_{ch}")
                             for ch in range(NQ // RCH)])
            for ch in range(NQ // RCH):
                for cc in range(CB):
                    cs = slice(cc * P, (cc + 1) * P)
                    nc.sync.dma_start(out=t_sb[cc][ch][:],
                                      in_=t[cs, ch * RCH:(ch + 1) * RCH])
            r8_sb = persist.tile([P, 2 * N], FP8, tag="r8")
            for ch in range(4):
                nc.sync.dma_start(out=r8_sb[:, ch * 2 * RCH:(ch + 1) * 2 * RCH],
                                  in_=r8[:, ch * 2 * RCH:(ch + 1) * 2 * RCH])
            for ch in range(N // RCH):
                for cc in range(CB):
                    cs = slice(cc * P, (cc + 1) * P)
                    nc.sync.dma_start(out=r_sb[cc][ch][:],
                                      in_=r[cs, ch * RCH:(ch + 1) * RCH])

            def r_slice(cc, jb):
                return r_sb[cc][(jb * P) // RCH][
                    :, (jb * P) % RCH:(jb * P) % RCH + P]

            # ---- projections ------------------------------------------------
            # u[b, i] = sum_a m[a, b] t[a, i]  (+g on the DVE copy);
            # stored fp8 in [c_lo, (b_hi, i)] layout for DoubleRow scores.
            u8 = persist.tile([P, 2 * NQ], FP8, tag="u8")
            for half in range(NQ // RCH):
                for bb in range(CB):
                    bs = slice(bb * P, (bb + 1) * P)
                    up = ps_s.tile([P, RCH], F32, tag="s", name="up")
                    for ac in range(CB):
                        for nch in range(2):
                            nc.tensor.matmul(
                                up[:, nch * 512:(nch + 1) * 512],
                                lhsT=m_sb[ac][:, bs],
                                rhs=t_sb[ac][half][:, nch * 512:
                                                   (nch + 1) * 512],
                                start=(ac == 0), stop=(ac == CB - 1),
                            )
                    nc.vector.tensor_scalar_add(
                        u8[:, bb * NQ + half * RCH:bb * NQ + (half + 1) * RCH],
                        up[:], g_sb[bb][:])

            # vT[j, c] = sum_c' r[c', j] wvT[c', c]  (r is the stationary op);
            # two key blocks share one PSUM tile and one DVE copy.
            v_sb = persist.tile([P, NJB * C], BF16, tag="v")
            for jp in range(NJB // 2):
                vp = ps_av.tile([P, 2 * C], F32, tag="av", name="vp")
                for j2 in range(2):
                    jb = 2 * jp + j2
                    for cc in range(CB):
                        nc.tensor.matmul(
                            vp[:, j2 * C:(j2 + 1) * C],
                            lhsT=r_slice(cc, jb),
                            rhs=wv_sb[cc][:],
                            start=(cc == 0), stop=(cc == CB - 1),
                        )
                nc.vector.tensor_copy(out=v_sb[:, jp * 2 * C:(jp + 1) * 2 * C],
                                      in_=vp[:])

            u3 = u8.rearrange("p (h q) -> p h q", h=2)

            # ---- attention: pairs of query chunks ---------------------------
            for icp in range(NICH // 2):
                av = [ps_av.tile([P, ICH], F32, tag="av", name=f"av{icp}_{k}")
                      for k in range(2 * CB)]  # index = cb * 2 + ic2
                ets = {}

                def emit_scores(jb, icp=icp, ets=ets):
                    sps = ps_s.tile([P, 2 * ICH], F32, tag="s", name="sps")
                    r8_ap = r8_sb[:, jb * 2 * P:(jb + 1) * 2 * P
                                  ].rearrange("p (h j) -> p h j", h=2)
                    for ic2 in range(2):
                        isl = slice((2 * icp + ic2) * ICH,
                                    (2 * icp + ic2 + 1) * ICH)
                        nc.tensor.matmul(
                            sps[:, ic2 * ICH:(ic2 + 1) * ICH],
                            lhsT=r8_ap,
                            rhs=u3[:, :, isl],
                            start=True, stop=True,
                            perf_mode=mybir.MatmulPerfMode.DoubleRow,
                        )
                    et = epool.tile([P, 2 * ICH], BF16, tag="e", name="et")
                    nc.scalar.activation(et[:], sps[:],
                                         mybir.ActivationFunctionType.Exp,
                                         scale=SCALE)
                    nc.sync.dma_start(
                        out=e_out[jb * P:(jb + 1) * P,
                                  icp * 2 * ICH:(icp + 1) * 2 * ICH],
                        in_=et[:])
                    ets[jb] = et

                def emit_av(jb, icp=icp, av=av, ets=ets, final=False):
                    et = ets.pop(jb)
                    for cb in range(CB):
                        for ic2 in range(2):
                            k = cb * 2 + ic2
                            nc.tensor.matmul(
                                av[k][:],
                                lhsT=v_sb[:, jb * C + cb * P:
                                          jb * C + (cb + 1) * P],
                                rhs=et[:, ic2 * ICH:(ic2 + 1) * ICH],
                                start=(jb == 0), stop=(jb == NJB - 1),
                            )
                            if final:
                                # evacuate PSUM right behind the last matmul,
                                # alternating engines so the 4 copies drain in
                                # ~2 copy-times and free the bank for the next
                                # query-chunk pair.
                                isl = slice((2 * icp + ic2) * ICH,
                                            (2 * icp + ic2 + 1) * ICH)
                                ot = outp.tile([P, ICH], F32, tag="o",
                                               name="ot")
                                if k % 2 == 0:
                                    nc.vector.tensor_copy(out=ot[:],
                                                          in_=av[k][:])
                                else:
                                    nc.scalar.copy(ot[:], av[k][:])
                                nc.sync.dma_start(
                                    out=o[cb * P:(cb + 1) * P, isl],
                                    in_=ot[:])

                emit_scores(0)
                for jb in range(1, NJB):
                    emit_scores(jb)
                    emit_av(jb - 1)
                emit_av(NJB - 1, final=True)

    nc.finalize()
    return nc


_NC_CACHE = None


def kernel(target, reference, Wq, bq, Wk, bk, Wv, bv):
    global _NC_CACHE, LAST_RESULTS
    target = np.asarray(target, np.float32)
    reference = np.asarray(reference, np.float32)
    Wq, Wk, Wv = (np.asarray(w, np.float32) for w in (Wq, Wk, Wv))
    bq, bk, bv = (np.asarray(b_, np.float32) for b_ in (bq, bk, bv))

    if _NC_CACHE is None:
        _NC_CACHE = _build()
    nc = _NC_CACHE

    t_full = target.reshape(B, C, N)
    r_full = reference.reshape(B, C, N)
    m_mat = (Wq.T @ Wk).astype(NPBF16)           # scores fold: M = Wq^T Wk
    g_vec = (Wk.T @ bq).reshape(C, 1)            # bq fold (bk cancels exactly)
    w_common = {
        "m": m_mat,
        "wvT": np.ascontiguousarray(Wv.T).astype(NPBF16),
        "g": g_vec,
    }
    in_maps = []
    for cid in range(NCORES):
        b_, h_ = cid // 2, cid % 2
        # r8: DoubleRow stationary layout [c_lo, (jb, c_hi, j_local)]
        r8 = (r_full[b_].reshape(CB, P, NJB, P)
              .transpose(1, 2, 0, 3).reshape(P, 2 * N))
        in_maps.append({
            "t": np.ascontiguousarray(
                t_full[b_][:, h_ * NQ:(h_ + 1) * NQ]).astype(NPBF16),
            "r": r_full[b_].astype(NPBF16),
            "r8": np.ascontiguousarray(r8).astype(NPFP8),
            **w_common,
        })

    res = run_bass_kernel_spmd(
        nc, in_maps, core_ids=list(range(NCORES)), trace=TRACE,
    )
    LAST_RESULTS = res

    out = np.empty((B, C, N), np.float32)
    for cid in range(NCORES):
        b_, h_ = cid // 2, cid % 2
        o = res.results[cid]["o"].astype(np.float64)
        den = res.results[cid]["e_out"].astype(np.float32).sum(
            axis=0, dtype=np.float64)
        sl = slice(h_ * NQ, (h_ + 1) * NQ)
        out[b_][:, sl] = (o / den[None, :] + bv.astype(np.float64)[:, None]
                          + t_full[b_][:, sl])
    return out.reshape(B, C, H, W)
